# revision 56
# baseline (speedup 1.0000x reference)
"""Trainium2 Bass kernel for the AttentiveRouter MoE-routing module.

Strategy (8 NeuronCores, data parallel over tokens):
  - core c gets x[c] : [4096, 2048] (batch dim == 8 == n_cores)
  - router weights replicated; w1 pre-scaled by ln_w and pre-transposed on
    host (weight layout prep only - all per-token compute is on device)
  - per core: LayerNorm -> Linear(2048->1024) -> exact GELU -> Linear(1024->8)
    -> clip(logits/T) -> top-2 + softmax -> dense mask -> row-normalize,
    plus per-expert column sums of the pre-norm mask
  - host: gathers shards, sums the 8 per-core [8] column-sum partials
    (the "all-reduce" of the capacity check), computes usage + KL loss.
    The capacity drop itself is inactive for this regime (col sums max
    ~7.5k vs capacity 16384); a numpy fallback handles the general case.

Compute layout per core (B-form: H2 on partitions for the big matmul):
  psum1[m, t] = sum_k w1s[k, m] * xc[t, k]   (xc = LayerNorm'd x, transposed
  on the TensorEngine in 128x128 blocks).

Precision: fp32 matmuls on TRN2 are decomposed into 2 HW passes at
2 cycles/column (4x the bf16/fp16 rate), so mm1 runs in fp16 (1 cy/col,
fp32 PSUM accumulation). mm2 feeds the top-2 selection, which is
sensitive to logit error, so w2 is split into fp16 hi + lo*2^12 halves
concatenated into one [1024, 40] stationary (lo lands at PSUM partition
32 for alignment); the epilogue recombines hi + lo*2^-12, making w2
effectively exact. Measured mask error vs the fp64 reference: 8.3e-3
(6 top-2 flips out of 32768 tokens), well inside the 2e-2 gate.
LayerNorm rstd uses a quake-style rsqrt + 2 Newton steps on the vector
engine (avoids ACT table swaps; Gelu stays resident all loop, Sigmoid
is batched once at the end).
"""

import math
import numpy as np

import concourse.bass as bass
import concourse.bacc as bacc
import concourse.mybir as mybir
import concourse.tile as tile
from concourse.bass_utils import run_bass_kernel_spmd

f32 = mybir.dt.float32
f16 = mybir.dt.float16
i32 = mybir.dt.int32
AF = mybir.ActivationFunctionType
ALU = mybir.AluOpType
AX = mybir.AxisListType

TEMP = 0.7
EPS = 1e-6
LN_EPS = 1e-5
CAP_FACTOR = 2.0

P = 128          # partitions
H = 2048         # hidden
H2 = 1024        # router mlp hidden
E = 8            # experts
NCH = 256        # tokens per chunk
KC = H // P      # 16 k-chunks
MT = H2 // P     # 8 m-tiles

_GRAPH_CACHE = {}


def build_graph(n_tokens: int, use_c: bool, n_cores: int = 8):
    """Builds the per-core Bass graph (SPMD: same graph, different shards)."""
    n_chunks = n_tokens // NCH
    n_tiles = n_tokens // P
    nc = bacc.Bacc(
        "TRN2",
        target_bir_lowering=False,
        debug=False,
        enable_asserts=False,
        num_devices=n_cores,
    )

    x_d = nc.dram_tensor("x", [n_tokens, H], f32, kind="ExternalInput").ap()
    w1s_d = nc.dram_tensor("w1s", [H, H2], f16, kind="ExternalInput").ap()
    w2hl_d = nc.dram_tensor("w2hl", [H2, 32 + E], f16, kind="ExternalInput").ap()
    b2s_d = nc.dram_tensor("b2s", [E, 1], f32, kind="ExternalInput").ap()
    ident_d = nc.dram_tensor("ident", [P, P], f32, kind="ExternalInput").ap()
    identh_d = nc.dram_tensor("identh", [P, P], f16, kind="ExternalInput").ap()
    if use_c:
        cvec_d = nc.dram_tensor("cvec", [H2, 1], f32, kind="ExternalInput").ap()
    ew_d = nc.dram_tensor("ew", [n_tokens, E], f32, kind="ExternalOutput").ap()
    mask_d = nc.dram_tensor("mask", [n_tokens, E], f32, kind="ExternalOutput").ap()
    cs_d = nc.dram_tensor("cs", [1, E], f32, kind="ExternalOutput").ap()

    with tile.TileContext(nc) as tc:
        from contextlib import ExitStack

        with ExitStack() as ctx:
            constp = ctx.enter_context(tc.tile_pool(name="const", bufs=1))
            xtp = ctx.enter_context(tc.tile_pool(name="xt", bufs=4))
            xcp = ctx.enter_context(tc.tile_pool(name="xc", bufs=4))
            xctp = ctx.enter_context(tc.tile_pool(name="xct", bufs=2))
            htp = ctx.enter_context(tc.tile_pool(name="ht", bufs=2))
            smp = ctx.enter_context(tc.tile_pool(name="sm", bufs=10))
            ewp = ctx.enter_context(tc.tile_pool(name="ewt", bufs=2))
            outp = ctx.enter_context(tc.tile_pool(name="outs", bufs=1))
            psp = ctx.enter_context(tc.tile_pool(name="psum", bufs=4, space="PSUM"))
            pstp = ctx.enter_context(tc.tile_pool(name="pstr", bufs=4, space="PSUM"))

            # ---- persistent constants ----
            # Loaded on the gpsimd (SWDGE) queue, and emitted after the first
            # chunk's x DMAs, so the ~0.6us-per-DMA issue cost never delays
            # the critical startup path on the sync queue.
            w1s_sb = constp.tile([P, KC, H2], f16)  # [p, kc, m] = w1s[kc*128+p, m]
            w2hl_sb = constp.tile([P, MT, 32 + E], f16)  # [p, kc2, (hi|pad|lo)]
            b2s_sb = constp.tile([E, 1], f32)
            ident_sb = constp.tile([P, P], f32)
            identh_sb = constp.tile([P, P], f16)
            ones_sb = constp.tile([P, 1], f32)
            c_sb = constp.tile([P, MT], f32) if use_c else None

            def load_weights():
                # small constants first: the identity matrices gate the very
                # first PE transposes, so they must not queue behind 4MB of w1s
                nc.gpsimd.dma_start(identh_sb[:], identh_d[:, :])
                nc.gpsimd.dma_start(ident_sb[:], ident_d[:, :])
                nc.gpsimd.dma_start(b2s_sb[:], b2s_d[:, :])
                nc.gpsimd.dma_start(
                    w2hl_sb[:], w2hl_d.rearrange("(kc p) e -> p kc e", p=P)
                )
                if use_c:
                    nc.gpsimd.dma_start(
                        c_sb[:], cvec_d.rearrange("(j p) o -> p (j o)", p=P)
                    )
                w1r = w1s_d.rearrange("(kc p) m -> p kc m", p=P)
                for kc in range(KC):
                    nc.gpsimd.dma_start(w1s_sb[:, kc, :], w1r[:, kc, :])
                nc.vector.memset(ones_sb[:], 1.0)

            # ---- persistent outputs (accumulated, DMA'd at the end) ----
            ew_all = outp.tile([P, n_tiles, E], f32)
            # tile t of chunk i holds tokens [(2i+t)*128, (2i+t+1)*128)

            def prep_stage1(i):
                """dma + stats + rstd + center for chunk i -> xc fp16 tiles."""
                xts = []
                for t in range(2):
                    xt = xtp.tile([P, H], f32, tag="xt")
                    r0 = (2 * i + t) * P
                    if i < 3:
                        # startup: halve transfer latency by pairing the
                        # sync and scalar HWDGE queues
                        nc.sync.dma_start(xt[:, 0:H // 2], x_d[r0:r0 + P, 0:H // 2])
                        nc.scalar.dma_start(
                            xt[:, H // 2:H], x_d[r0:r0 + P, H // 2:H]
                        )
                    else:
                        nc.sync.dma_start(xt[:], x_d[r0:r0 + P, :])
                    xts.append(xt)

                # bn_stats per 512-seg -> mean/var per tile
                mv = smp.tile([P, 2, 2], f32, tag="mv")  # [p, t, (mean, var)]
                for t in range(2):
                    bno = smp.tile([P, 4, 6], f32, tag="bno")
                    for seg in range(4):
                        nc.vector.bn_stats(
                            bno[:, seg, :], xts[t][:, seg * 512:(seg + 1) * 512]
                        )
                    nc.vector.bn_aggr(mv[:, t, :], bno[:, :, :])

                mu = mv[:, :, 0]      # [128, 2] stride-2 AP
                var = mv[:, :, 1]

                # rstd = rsqrt(var + eps): quake guess on vhalf (magic
                # adjusted by -2^22 for the /2), then 2 Newton steps
                # (rel err ~5e-6).
                g = nc.vector
                vhalf = smp.tile([P, 2], f32, tag="vhalf")
                g.tensor_scalar(vhalf[:], var, LN_EPS, 0.5, ALU.add, ALU.mult)
                yt = smp.tile([P, 2], f32, tag="yt")
                sh = smp.tile([P, 2], i32, tag="sh")
                g.tensor_scalar(
                    sh[:], vhalf[:].bitcast(i32), 1, None, ALU.arith_shift_right
                )
                g.tensor_scalar(
                    yt[:].bitcast(i32), sh[:], -1, 0x5EF759DF, ALU.mult, ALU.add
                )
                for _ in range(2):
                    yy = smp.tile([P, 2], f32, tag="yy")
                    g.tensor_mul(yy[:], yt[:], yt[:])
                    g.tensor_mul(yy[:], yy[:], vhalf[:])
                    g.tensor_scalar(yy[:], yy[:], -1.0, 1.5, ALU.mult, ALU.add)
                    y2 = smp.tile([P, 2], f32, tag="yt")
                    g.tensor_mul(y2[:], yt[:], yy[:])
                    yt = y2
                # neg(mu * rstd) = (mu * -1) * rstd
                nmr = smp.tile([P, 2], f32, tag="nmr")
                g.scalar_tensor_tensor(nmr[:], mu, -1.0, yt[:], ALU.mult, ALU.mult)

                # center+scale (fp32 -> fp16)
                xcs = []
                for t in range(2):
                    xc = xcp.tile([P, H], f16, tag="xc")
                    nc.scalar.activation(
                        xc[:], xts[t][:],
                        AF.Identity,
                        bias=nmr[:, t:t + 1],
                        scale=yt[:, t:t + 1],
                    )
                    xcs.append(xc)
                return xcs

            def prep_stage2(i, xcs):
                """transpose 128x128 blocks on PE -> xcT buffer."""
                xct = xctp.tile([P, KC, NCH], f16, tag="xct")
                for t in range(2):
                    for g in range(4):
                        pst = pstp.tile([P, 512], f16, tag="pst")
                        for b in range(4):
                            kc = g * 4 + b
                            nc.tensor.transpose(
                                pst[:, b * P:(b + 1) * P],
                                xcs[t][:, kc * P:(kc + 1) * P],
                                identh_sb[:],
                            )
                        nc.vector.tensor_copy(
                            xct[:, g * 4:(g + 1) * 4, t * P:(t + 1) * P],
                            pst[:].rearrange("p (b q) -> p b q", b=4),
                        )
                return xct

            def compute_mm1(i, xct):
                """mm1 + gelu -> hT (fp16)."""
                hT = htp.tile([P, MT, NCH], f16, tag="ht")
                for j in range(MT // 2):
                    ps = psp.tile([P, 512], f32, tag="ps")
                    for half in range(2):
                        m = 2 * j + half
                        for k in range(KC):
                            nc.tensor.matmul(
                                ps[:, half * NCH:(half + 1) * NCH],
                                lhsT=w1s_sb[:, k, m * P:(m + 1) * P],
                                rhs=xct[:, k, :],
                                start=(k == 0),
                                stop=(k == KC - 1),
                            )
                    if use_c:
                        for half in range(2):
                            m = 2 * j + half
                            nc.scalar.activation(
                                hT[:, m, :], ps[:, half * NCH:(half + 1) * NCH],
                                AF.Gelu, bias=c_sb[:, m:m + 1],
                            )
                    else:
                        nc.scalar.activation(
                            hT[:, 2 * j:2 * j + 2, :],
                            ps[:].rearrange("p (a q) -> p a q", a=2),
                            AF.Gelu,
                        )
                return hT

            def compute_tail(i, hT):
                """mm2 + clip + transpose-to-token-major."""
                # ps2 (cols 0-255, partitions 0-39) and psE (cols 496-511)
                # share one PSUM bank
                psc = pstp.tile([P, 512], f32, tag="pst")
                ps2 = psc[0:32 + E, 0:NCH]
                for k2 in range(MT):
                    nc.tensor.matmul(
                        ps2[:],
                        lhsT=w2hl_sb[:, k2, :],
                        rhs=hT[:, k2, :],
                        start=(k2 == 0),
                        stop=(k2 == MT - 1),
                    )
                # ew = clip((logits + b2)/T, -50, 50)   [E, NCH]
                # logits = psum_hi + psum_lo * 2^-12 ; only one PSUM operand
                # per DVE op (s2s2d2 ISA constraint).
                ewt = ewp.tile([E, NCH], f32, tag="ewt")
                nc.vector.tensor_scalar(
                    ewt[:], psc[0:E, 0:NCH], b2s_sb[:], 1.0 / TEMP,
                    ALU.add, ALU.mult,
                )
                nc.vector.scalar_tensor_tensor(
                    ewt[:], psc[32:32 + E, 0:NCH], 1.0 / (4096.0 * TEMP), ewt[:],
                    ALU.mult, ALU.add,
                )
                nc.vector.tensor_scalar(
                    ewt[:], ewt[:], 50.0, -50.0, ALU.min, ALU.max
                )
                # transpose to token-major [128, E] blocks
                psE = psc[:, 496:496 + 2 * E]
                for t in range(2):
                    nc.tensor.transpose(
                        psE[:, t * E:(t + 1) * E],
                        ewt[:, t * P:(t + 1) * P],
                        ident_sb[:E, :E],
                    )
                nc.vector.tensor_copy(
                    ew_all[:, 2 * i:2 * i + 2, :],
                    psE.rearrange("p (t e) -> p t e", t=2),
                )

            # routing tiles (filled in halves while the loop still runs)
            m1 = outp.tile([P, n_tiles], f32)
            flag1 = outp.tile([P, n_tiles, E], f32)
            masked = outp.tile([P, n_tiles, E], f32)
            m2 = outp.tile([P, n_tiles], f32)
            flag2 = outp.tile([P, n_tiles, E], f32)
            d = outp.tile([P, n_tiles], f32)
            ew_dr = ew_d.rearrange("(n p) e -> p n e", p=P)

            def routing_pre(h):
                """top-2 flags for half h of the tiles (sigmoid-free part)."""
                sl = slice(h * n_tiles // 2, (h + 1) * n_tiles // 2)
                nt = n_tiles // 2
                ewv = ew_all[:, sl, :]
                nc.vector.reduce_max(m1[:, sl], ewv, axis=AX.X)
                nc.vector.tensor_tensor(
                    flag1[:, sl, :], ewv, m1[:, sl].broadcast_to([P, nt, E]),
                    ALU.is_ge,
                )
                nc.vector.scalar_tensor_tensor(
                    masked[:, sl, :], flag1[:, sl, :], -1e30, ewv,
                    ALU.mult, ALU.add,
                )
                nc.vector.reduce_max(m2[:, sl], masked[:, sl, :], axis=AX.X)
                nc.vector.tensor_tensor(
                    flag2[:, sl, :], masked[:, sl, :],
                    m2[:, sl].broadcast_to([P, nt, E]), ALU.is_ge,
                )
                nc.vector.tensor_sub(d[:, sl], m2[:, sl], m1[:, sl])

            # ---- main pipeline ----
            # stage1 runs one chunk ahead of the PE transposes so the
            # center pass never head-of-line-blocks the PE.
            xcs_q = []
            xct_q = []
            for i in range(n_chunks + 2):
                if i == 2:
                    # first compute ahead of T(1): PE starts mm1(0) right
                    # after T(0) instead of stalling on center(1)
                    hT = compute_mm1(0, xct_q.pop(0))
                    compute_tail(0, hT)
                # stage2 next: its DVE copies must precede the next
                # chunk's stats/chain in the DVE queue
                if 1 <= i <= n_chunks:
                    xct_q.append(prep_stage2(i - 1, xcs_q.pop(0)))
                if i < n_chunks:
                    xcs_q.append(prep_stage1(i))
                if i == 0:
                    load_weights()
                if i >= 3:
                    hT = compute_mm1(i - 2, xct_q.pop(0))
                    compute_tail(i - 2, hT)
                    c = i - 2
                    if (c + 1) % 4 == 0:
                        # stream the finished ew rows out during the loop
                        nc.sync.dma_start(
                            ew_dr[:, 2 * c - 6:2 * c + 2, :],
                            ew_all[:, 2 * c - 6:2 * c + 2, :],
                        )
                        ew_streamed = 2 * c + 2
                    if c == max(n_chunks // 2 - 1, 0) and n_chunks > 1:
                        routing_pre(0)

            if n_chunks == 1:
                routing_pre(0)
            if (ew_streamed if n_chunks >= 4 else 0) < n_tiles:
                lo = ew_streamed if n_chunks >= 4 else 0
                nc.sync.dma_start(ew_dr[:, lo:, :], ew_all[:, lo:, :])
            routing_pre(1)
            s2 = outp.tile([P, n_tiles], f32)
            nc.scalar.activation(s2[:], d[:], AF.Sigmoid)
            s1 = outp.tile([P, n_tiles], f32)
            nc.vector.tensor_scalar(s1[:], s2[:], -1.0, 1.0, ALU.mult, ALU.add)

            # mask assembly / colsum / rownorm / DMA in halves so the PE
            # colsum matmul and the mask DMA overlap the DVE chain
            mask_pre = outp.tile([P, n_tiles, E], f32)
            flag2w = outp.tile([P, n_tiles, E], f32)
            rowsum = outp.tile([P, n_tiles], f32)
            rinv = outp.tile([P, n_tiles], f32)
            csp = psp.tile([1, n_tiles * E], f32, tag="ps")
            mask_dr = mask_d.rearrange("(n p) e -> p n e", p=P)
            nt2 = n_tiles // 2
            for hh in range(2):
                sl = slice(hh * nt2, (hh + 1) * nt2)
                nc.vector.tensor_tensor(
                    mask_pre[:, sl, :], flag1[:, sl, :],
                    s1[:, sl].broadcast_to([P, nt2, E]), ALU.mult,
                )
                nc.vector.tensor_tensor(
                    flag2w[:, sl, :], flag2[:, sl, :],
                    s2[:, sl].broadcast_to([P, nt2, E]), ALU.mult,
                )
                nc.vector.tensor_add(
                    mask_pre[:, sl, :], mask_pre[:, sl, :], flag2w[:, sl, :]
                )
                nc.tensor.matmul(
                    csp[:, hh * nt2 * E:(hh + 1) * nt2 * E],
                    lhsT=ones_sb[:],
                    rhs=mask_pre[:, sl, :].rearrange("p n e -> p (n e)"),
                    start=True,
                    stop=True,
                )
                nc.vector.reduce_sum(
                    rowsum[:, sl], mask_pre[:, sl, :], axis=AX.X
                )
                nc.vector.tensor_scalar(
                    rowsum[:, sl], rowsum[:, sl], EPS, None, ALU.max
                )
                nc.vector.reciprocal(rinv[:, sl], rowsum[:, sl])
                nc.vector.tensor_tensor(
                    mask_pre[:, sl, :], mask_pre[:, sl, :],
                    rinv[:, sl].broadcast_to([P, nt2, E]), ALU.mult,
                )
                nc.sync.dma_start(mask_dr[:, sl, :], mask_pre[:, sl, :])

            cs_sb = outp.tile([1, E], f32)
            nc.vector.reduce_sum(
                cs_sb[:], csp[:].rearrange("p (n e) -> p e n", e=E), axis=AX.X
            )
            nc.sync.dma_start(cs_d[:, :], cs_sb[:])

    nc.compile()
    return nc


def _get_graph(n_tokens, use_c, n_cores=8):
    key = (n_tokens, use_c, n_cores)
    if key not in _GRAPH_CACHE:
        _GRAPH_CACHE[key] = build_graph(n_tokens, use_c, n_cores)
    return _GRAPH_CACHE[key]


def _host_fallback(x, ln_w, ln_b, w1, b1, w2, b2, top_k, num_experts):
    """Full numpy reference path; only used if the capacity drop binds."""
    B, S, Hd = x.shape
    N = B * S
    mu = x.mean(-1, keepdims=True)
    var = ((x - mu) ** 2).mean(-1, keepdims=True)
    h = (x - mu) / np.sqrt(var + LN_EPS) * ln_w + ln_b
    pre = h @ w1.T + b1
    from numpy import vectorize
    from math import erf as _erf
    h2 = pre * 0.5 * (1.0 + np.vectorize(_erf)(pre / math.sqrt(2.0)))
    logits = h2 @ w2.T + b2
    ew = np.clip(logits / TEMP, -50.0, 50.0).astype(np.float32)
    flat = ew.reshape(N, num_experts)
    capacity = int(CAP_FACTOR * N * top_k / num_experts)
    tk_i = np.argsort(-flat, axis=-1, kind="stable")[:, :top_k]
    tk_w = np.take_along_axis(flat, tk_i, axis=-1)
    ex = np.exp(tk_w - tk_w.max(-1, keepdims=True))
    sm = (ex / ex.sum(-1, keepdims=True)).astype(np.float32)
    masks = np.zeros((N, num_experts), np.float32)
    np.put_along_axis(masks, tk_i, sm, axis=-1)
    cols = masks.T.copy()
    sums = cols.sum(-1)
    kcap = min(capacity, N)
    over = sums > capacity
    for e in np.where(over)[0]:
        order = np.argsort(-cols[e], kind="stable")
        keep = order[:kcap]
        ncol = np.zeros_like(cols[e])
        ncol[keep] = cols[e][keep]
        cols[e] = ncol
    count = np.where(over, cols.sum(-1), sums)
    masks = cols.T
    masks = masks / np.maximum(masks.sum(-1, keepdims=True), EPS)
    usage = (count / np.maximum(count.sum(), EPS)).astype(np.float32)
    target = np.full((num_experts,), 1.0 / num_experts, np.float32)
    kl = np.sum(target * (np.log(target) - np.log(np.maximum(usage, EPS)))) / num_experts
    loss = np.float32(0.01 * kl)
    return (
        ew.reshape(B, S, num_experts),
        masks.reshape(B, S, num_experts).astype(np.float32),
        loss,
        usage,
    )


def _prep_inputs(x, ln_w, ln_b, w1, b1, w2, b2, n_cores=8):
    """Host-side weight layout prep + sharding."""
    w1s = np.ascontiguousarray((w1 * ln_w[None, :]).T).astype(np.float16)  # [H, H2]
    cvec = (b1 + w1 @ ln_b).astype(np.float32)                        # [H2]
    use_c = bool(np.any(cvec))
    w2s = np.ascontiguousarray(w2.T, np.float32)                      # [H2, E]
    w2h = w2s.astype(np.float16)
    w2l = ((w2s - w2h.astype(np.float32)) * 4096.0).astype(np.float16)
    w2hl = np.ascontiguousarray(np.concatenate(
        [w2h, np.zeros((w2h.shape[0], 24), np.float16), w2l], axis=1))
    b2s = np.ascontiguousarray(b2.reshape(E, 1), np.float32)
    ident = np.eye(P, dtype=np.float32)
    xs = x.reshape(n_cores, -1, H)
    in_maps = []
    for c in range(n_cores):
        m = {
            "x": np.ascontiguousarray(xs[c], np.float32),
            "w1s": w1s,
            "w2hl": w2hl,
            "b2s": b2s,
            "ident": ident,
            "identh": ident.astype(np.float16),
        }
        if use_c:
            m["cvec"] = cvec.reshape(H2, 1)
        in_maps.append(m)
    return in_maps, use_c


def _finalize(results, B, S, n_cores=8):
    """Gather shards; global column-sum reduce; usage + loss on host."""
    N = B * S
    num_experts = E
    ew = np.stack([results[c]["ew"] for c in range(n_cores)]).reshape(B, S, E)
    masks = np.stack([results[c]["mask"] for c in range(n_cores)]).reshape(B, S, E)
    cs = np.stack([results[c]["cs"][0] for c in range(n_cores)]).sum(axis=0)
    count = cs.astype(np.float32)
    usage = (count / np.maximum(count.sum(), EPS)).astype(np.float32)
    target = np.full((num_experts,), 1.0 / num_experts, np.float32)
    kl = np.sum(
        target * (np.log(target) - np.log(np.maximum(usage, EPS)))
    ) / num_experts
    loss = np.float32(0.01 * kl)
    return ew, masks, loss, usage, cs


def run(x, ln_w, ln_b, w1, b1, w2, b2, top_k, num_experts, trace=False):
    assert int(top_k) == 2 and int(num_experts) == E
    B, S, Hd = x.shape
    assert Hd == H and B == 8 and S == 4096, (B, S, Hd)
    n_cores = 8
    n_tokens = B * S // n_cores
    x = np.asarray(x, np.float32)
    in_maps, use_c = _prep_inputs(
        x, np.asarray(ln_w, np.float32), np.asarray(ln_b, np.float32),
        np.asarray(w1, np.float32), np.asarray(b1, np.float32),
        np.asarray(w2, np.float32), np.asarray(b2, np.float32), n_cores
    )
    nc = _get_graph(n_tokens, use_c, n_cores)
    res = run_bass_kernel_spmd(
        nc, in_maps, core_ids=list(range(n_cores)), trace=trace
    )
    ew, masks, loss, usage, cs = _finalize(res.results, B, S, n_cores)
    capacity = int(CAP_FACTOR * B * S * int(top_k) / int(num_experts))
    if (cs > capacity).any():
        return _host_fallback(
            x, ln_w, ln_b, w1, b1, w2, b2, int(top_k), int(num_experts)
        ), res
    return (ew, masks, loss, usage), res


def kernel(x, ln_w, ln_b, w1, b1, w2, b2, top_k, num_experts):
    out, _ = run(x, ln_w, ln_b, w1, b1, w2, b2, top_k, num_experts)
    return out


# revision 57
# speedup vs baseline: 1.2201x; 1.2201x over previous
"""Trainium2 Bass kernel for the AttentiveRouter MoE-routing module.

Strategy (8 NeuronCores, data parallel over tokens):
  - core c gets x[c] : [4096, 2048] (batch dim == 8 == n_cores)
  - router weights replicated; w1 pre-scaled by ln_w and pre-transposed on
    host (weight layout prep only - all per-token compute is on device)
  - per core: LayerNorm -> Linear(2048->1024) -> exact GELU -> Linear(1024->8)
    -> clip(logits/T) -> top-2 + softmax -> dense mask -> row-normalize,
    plus per-expert column sums of the pre-norm mask
  - host: gathers shards, sums the 8 per-core [8] column-sum partials
    (the "all-reduce" of the capacity check), computes usage + KL loss.
    The capacity drop itself is inactive for this regime (col sums max
    ~7.5k vs capacity 16384); a numpy fallback handles the general case.

Compute layout per core (B-form: H2 on partitions for the big matmul):
  psum1[m, t] = sum_k w1s[k, m] * xc[t, k]   (xc = LayerNorm'd x, transposed
  on the TensorEngine in 128x128 blocks).

Precision: fp32 matmuls on TRN2 are decomposed into 2 HW passes at
2 cycles/column (4x the bf16/fp16 rate), so mm1 runs in fp16 (1 cy/col,
fp32 PSUM accumulation). mm2 feeds the top-2 selection, which is
sensitive to logit error, so w2 is split into fp16 hi + lo*2^12 halves
concatenated into one [1024, 40] stationary (lo lands at PSUM partition
32 for alignment); the epilogue recombines hi + lo*2^-12, making w2
effectively exact. Measured mask error vs the fp64 reference: 8.3e-3
(6 top-2 flips out of 32768 tokens), well inside the 2e-2 gate.
LayerNorm rstd uses a quake-style rsqrt + 2 Newton steps on the vector
engine (avoids ACT table swaps; Gelu stays resident all loop, Sigmoid
is batched once at the end).
"""

import math
import numpy as np

import concourse.bass as bass
import concourse.bacc as bacc
import concourse.mybir as mybir
import concourse.tile as tile
from concourse.bass_utils import run_bass_kernel_spmd

f32 = mybir.dt.float32
f16 = mybir.dt.float16
i32 = mybir.dt.int32
AF = mybir.ActivationFunctionType
ALU = mybir.AluOpType
AX = mybir.AxisListType

TEMP = 0.7
EPS = 1e-6
LN_EPS = 1e-5
CAP_FACTOR = 2.0

P = 128          # partitions
H = 2048         # hidden
H2 = 1024        # router mlp hidden
E = 8            # experts
NCH = 256        # tokens per chunk
KC = H // P      # 16 k-chunks
MT = H2 // P     # 8 m-tiles

_GRAPH_CACHE = {}


def build_graph(n_tokens: int, use_c: bool, n_cores: int = 8):
    """Builds the per-core Bass graph (SPMD: same graph, different shards)."""
    n_chunks = n_tokens // NCH
    n_tiles = n_tokens // P
    nc = bacc.Bacc(
        "TRN2",
        target_bir_lowering=False,
        debug=False,
        enable_asserts=False,
        num_devices=n_cores,
    )

    x_d = nc.dram_tensor("x", [n_tokens, H], f32, kind="ExternalInput").ap()
    w1s_d = nc.dram_tensor("w1s", [H, H2], f16, kind="ExternalInput").ap()
    w2hl_d = nc.dram_tensor("w2hl", [H2, 32 + E], f16, kind="ExternalInput").ap()
    b2s_d = nc.dram_tensor("b2s", [E, 1], f32, kind="ExternalInput").ap()
    ident_d = nc.dram_tensor("ident", [P, P], f32, kind="ExternalInput").ap()
    identh_d = nc.dram_tensor("identh", [P, P], f16, kind="ExternalInput").ap()
    if use_c:
        cvec_d = nc.dram_tensor("cvec", [H2, 1], f32, kind="ExternalInput").ap()
    ew_d = nc.dram_tensor("ew", [n_tokens, E], f32, kind="ExternalOutput").ap()
    mask_d = nc.dram_tensor("mask", [n_tokens, E], f32, kind="ExternalOutput").ap()
    cs_d = nc.dram_tensor("cs", [1, E], f32, kind="ExternalOutput").ap()

    with tile.TileContext(nc) as tc:
        from contextlib import ExitStack

        with ExitStack() as ctx:
            constp = ctx.enter_context(tc.tile_pool(name="const", bufs=1))
            xtp = ctx.enter_context(tc.tile_pool(name="xt", bufs=4))
            xcp = ctx.enter_context(tc.tile_pool(name="xc", bufs=4))
            xctp = ctx.enter_context(tc.tile_pool(name="xct", bufs=2))
            htp = ctx.enter_context(tc.tile_pool(name="ht", bufs=2))
            smp = ctx.enter_context(tc.tile_pool(name="sm", bufs=10))
            ewp = ctx.enter_context(tc.tile_pool(name="ewt", bufs=2))
            outp = ctx.enter_context(tc.tile_pool(name="outs", bufs=1))
            psp = ctx.enter_context(tc.tile_pool(name="psum", bufs=4, space="PSUM"))
            pstp = ctx.enter_context(tc.tile_pool(name="pstr", bufs=4, space="PSUM"))

            # ---- persistent constants ----
            # Loaded on the gpsimd (SWDGE) queue, and emitted after the first
            # chunk's x DMAs, so the ~0.6us-per-DMA issue cost never delays
            # the critical startup path on the sync queue.
            w1s_sb = constp.tile([P, KC, H2], f16)  # [p, kc, m] = w1s[kc*128+p, m]
            w2hl_sb = constp.tile([P, MT, 32 + E], f16)  # [p, kc2, (hi|pad|lo)]
            b2s_sb = constp.tile([E, 1], f32)
            ident_sb = constp.tile([P, P], f32)
            identh_sb = constp.tile([P, P], f16)
            ones_sb = constp.tile([P, 1], f32)
            c_sb = constp.tile([P, MT], f32) if use_c else None

            def load_weights():
                # small constants first: the identity matrices gate the very
                # first PE transposes, so they must not queue behind 4MB of w1s
                nc.gpsimd.dma_start(identh_sb[:], identh_d[:, :])
                nc.gpsimd.dma_start(ident_sb[:], ident_d[:, :])
                nc.gpsimd.dma_start(b2s_sb[:], b2s_d[:, :])
                nc.gpsimd.dma_start(
                    w2hl_sb[:], w2hl_d.rearrange("(kc p) e -> p kc e", p=P)
                )
                if use_c:
                    nc.gpsimd.dma_start(
                        c_sb[:], cvec_d.rearrange("(j p) o -> p (j o)", p=P)
                    )
                w1r = w1s_d.rearrange("(kc p) m -> p kc m", p=P)
                for kc in range(KC):
                    nc.gpsimd.dma_start(w1s_sb[:, kc, :], w1r[:, kc, :])
                nc.vector.memset(ones_sb[:], 1.0)

            # ---- persistent outputs (accumulated, DMA'd at the end) ----
            ew_all = outp.tile([P, n_tiles, E], f32)
            # tile t of chunk i holds tokens [(2i+t)*128, (2i+t+1)*128)

            def prep_stage1(i):
                """dma + stats + rstd + center for chunk i -> xc fp16 tiles."""
                xts = []
                for t in range(2):
                    xt = xtp.tile([P, H], f32, tag="xt")
                    r0 = (2 * i + t) * P
                    nc.sync.dma_start(xt[:], x_d[r0:r0 + P, :])
                    xts.append(xt)

                # bn_stats per 512-seg -> mean/var per tile
                mv = smp.tile([P, 2, 2], f32, tag="mv")  # [p, t, (mean, var)]
                for t in range(2):
                    bno = smp.tile([P, 4, 6], f32, tag="bno")
                    for seg in range(4):
                        nc.vector.bn_stats(
                            bno[:, seg, :], xts[t][:, seg * 512:(seg + 1) * 512]
                        )
                    nc.vector.bn_aggr(mv[:, t, :], bno[:, :, :])

                mu = mv[:, :, 0]      # [128, 2] stride-2 AP
                var = mv[:, :, 1]

                # rstd = rsqrt(var + eps): quake guess on vhalf (magic
                # adjusted by -2^22 for the /2), then 2 Newton steps
                # (rel err ~5e-6).
                g = nc.vector
                vhalf = smp.tile([P, 2], f32, tag="vhalf")
                g.tensor_scalar(vhalf[:], var, LN_EPS, 0.5, ALU.add, ALU.mult)
                yt = smp.tile([P, 2], f32, tag="yt")
                sh = smp.tile([P, 2], i32, tag="sh")
                g.tensor_scalar(
                    sh[:], vhalf[:].bitcast(i32), 1, None, ALU.arith_shift_right
                )
                g.tensor_scalar(
                    yt[:].bitcast(i32), sh[:], -1, 0x5EF759DF, ALU.mult, ALU.add
                )
                for _ in range(2):
                    yy = smp.tile([P, 2], f32, tag="yy")
                    g.tensor_mul(yy[:], yt[:], yt[:])
                    g.tensor_mul(yy[:], yy[:], vhalf[:])
                    g.tensor_scalar(yy[:], yy[:], -1.0, 1.5, ALU.mult, ALU.add)
                    y2 = smp.tile([P, 2], f32, tag="yt")
                    g.tensor_mul(y2[:], yt[:], yy[:])
                    yt = y2
                # neg(mu * rstd) = (mu * -1) * rstd
                nmr = smp.tile([P, 2], f32, tag="nmr")
                g.scalar_tensor_tensor(nmr[:], mu, -1.0, yt[:], ALU.mult, ALU.mult)

                # center+scale (fp32 -> fp16)
                xcs = []
                for t in range(2):
                    xc = xcp.tile([P, H], f16, tag="xc")
                    nc.scalar.activation(
                        xc[:], xts[t][:],
                        AF.Identity,
                        bias=nmr[:, t:t + 1],
                        scale=yt[:, t:t + 1],
                    )
                    xcs.append(xc)
                return xcs

            def prep_stage2(i, xcs):
                """transpose 128x128 blocks on PE -> xcT buffer."""
                xct = xctp.tile([P, KC, NCH], f16, tag="xct")
                for t in range(2):
                    for g in range(4):
                        pst = pstp.tile([P, 512], f16, tag="pst")
                        for b in range(4):
                            kc = g * 4 + b
                            nc.tensor.transpose(
                                pst[:, b * P:(b + 1) * P],
                                xcs[t][:, kc * P:(kc + 1) * P],
                                identh_sb[:],
                            )
                        nc.vector.tensor_copy(
                            xct[:, g * 4:(g + 1) * 4, t * P:(t + 1) * P],
                            pst[:].rearrange("p (b q) -> p b q", b=4),
                        )
                return xct

            def compute_mm1(i, xct):
                """mm1 + gelu -> hT (fp16)."""
                hT = htp.tile([P, MT, NCH], f16, tag="ht")
                for j in range(MT // 2):
                    ps = psp.tile([P, 512], f32, tag="ps")
                    for half in range(2):
                        m = 2 * j + half
                        for k in range(KC):
                            nc.tensor.matmul(
                                ps[:, half * NCH:(half + 1) * NCH],
                                lhsT=w1s_sb[:, k, m * P:(m + 1) * P],
                                rhs=xct[:, k, :],
                                start=(k == 0),
                                stop=(k == KC - 1),
                            )
                    if use_c:
                        for half in range(2):
                            m = 2 * j + half
                            nc.scalar.activation(
                                hT[:, m, :], ps[:, half * NCH:(half + 1) * NCH],
                                AF.Gelu, bias=c_sb[:, m:m + 1],
                            )
                    else:
                        nc.scalar.activation(
                            hT[:, 2 * j:2 * j + 2, :],
                            ps[:].rearrange("p (a q) -> p a q", a=2),
                            AF.Gelu,
                        )
                return hT

            def compute_tail(i, hT):
                """mm2 + clip + transpose-to-token-major."""
                # ps2 (cols 0-255, partitions 0-39) and psE (cols 496-511)
                # share one PSUM bank
                psc = pstp.tile([P, 512], f32, tag="pst")
                ps2 = psc[0:32 + E, 0:NCH]
                for k2 in range(MT):
                    nc.tensor.matmul(
                        ps2[:],
                        lhsT=w2hl_sb[:, k2, :],
                        rhs=hT[:, k2, :],
                        start=(k2 == 0),
                        stop=(k2 == MT - 1),
                    )
                # ew = clip((logits + b2)/T, -50, 50)   [E, NCH]
                # logits = psum_hi + psum_lo * 2^-12 ; only one PSUM operand
                # per DVE op (s2s2d2 ISA constraint).
                ewt = ewp.tile([E, NCH], f32, tag="ewt")
                nc.vector.tensor_scalar(
                    ewt[:], psc[0:E, 0:NCH], b2s_sb[:], 1.0 / TEMP,
                    ALU.add, ALU.mult,
                )
                nc.vector.scalar_tensor_tensor(
                    ewt[:], psc[32:32 + E, 0:NCH], 1.0 / (4096.0 * TEMP), ewt[:],
                    ALU.mult, ALU.add,
                )
                nc.vector.tensor_scalar(
                    ewt[:], ewt[:], 50.0, -50.0, ALU.min, ALU.max
                )
                # transpose to token-major [128, E] blocks
                psE = psc[:, 496:496 + 2 * E]
                for t in range(2):
                    nc.tensor.transpose(
                        psE[:, t * E:(t + 1) * E],
                        ewt[:, t * P:(t + 1) * P],
                        ident_sb[:E, :E],
                    )
                nc.vector.tensor_copy(
                    ew_all[:, 2 * i:2 * i + 2, :],
                    psE.rearrange("p (t e) -> p t e", t=2),
                )

            # routing tiles (filled in halves while the loop still runs)
            m1 = outp.tile([P, n_tiles], f32)
            flag1 = outp.tile([P, n_tiles, E], f32)
            masked = outp.tile([P, n_tiles, E], f32)
            m2 = outp.tile([P, n_tiles], f32)
            flag2 = outp.tile([P, n_tiles, E], f32)
            d = outp.tile([P, n_tiles], f32)
            ew_dr = ew_d.rearrange("(n p) e -> p n e", p=P)

            def routing_pre(h):
                """top-2 flags for half h of the tiles (sigmoid-free part)."""
                sl = slice(h * n_tiles // 2, (h + 1) * n_tiles // 2)
                nt = n_tiles // 2
                ewv = ew_all[:, sl, :]
                nc.vector.reduce_max(m1[:, sl], ewv, axis=AX.X)
                nc.vector.tensor_tensor(
                    flag1[:, sl, :], ewv, m1[:, sl].broadcast_to([P, nt, E]),
                    ALU.is_ge,
                )
                nc.vector.scalar_tensor_tensor(
                    masked[:, sl, :], flag1[:, sl, :], -1e30, ewv,
                    ALU.mult, ALU.add,
                )
                nc.vector.reduce_max(m2[:, sl], masked[:, sl, :], axis=AX.X)
                nc.vector.tensor_tensor(
                    flag2[:, sl, :], masked[:, sl, :],
                    m2[:, sl].broadcast_to([P, nt, E]), ALU.is_ge,
                )
                nc.vector.tensor_sub(d[:, sl], m2[:, sl], m1[:, sl])

            # ---- main pipeline ----
            # stage1 runs one chunk ahead of the PE transposes so the
            # center pass never head-of-line-blocks the PE.
            xcs_q = []
            xct_q = []
            for i in range(n_chunks + 2):
                if i == 2:
                    # first compute ahead of T(1): PE starts mm1(0) right
                    # after T(0) instead of stalling on center(1)
                    hT = compute_mm1(0, xct_q.pop(0))
                    compute_tail(0, hT)
                # stage2 next: its DVE copies must precede the next
                # chunk's stats/chain in the DVE queue
                if 1 <= i <= n_chunks:
                    xct_q.append(prep_stage2(i - 1, xcs_q.pop(0)))
                if i < n_chunks:
                    xcs_q.append(prep_stage1(i))
                if i == 0:
                    load_weights()
                if i >= 3:
                    hT = compute_mm1(i - 2, xct_q.pop(0))
                    compute_tail(i - 2, hT)
                    c = i - 2
                    if (c + 1) % 4 == 0:
                        # stream the finished ew rows out during the loop
                        nc.sync.dma_start(
                            ew_dr[:, 2 * c - 6:2 * c + 2, :],
                            ew_all[:, 2 * c - 6:2 * c + 2, :],
                        )
                        ew_streamed = 2 * c + 2
                    if c == max(n_chunks // 2 - 1, 0) and n_chunks > 1:
                        routing_pre(0)

            if n_chunks == 1:
                routing_pre(0)
            if (ew_streamed if n_chunks >= 4 else 0) < n_tiles:
                lo = ew_streamed if n_chunks >= 4 else 0
                nc.sync.dma_start(ew_dr[:, lo:, :], ew_all[:, lo:, :])
            routing_pre(1)
            s2 = outp.tile([P, n_tiles], f32)
            nc.scalar.activation(s2[:], d[:], AF.Sigmoid)
            s1 = outp.tile([P, n_tiles], f32)
            nc.vector.tensor_scalar(s1[:], s2[:], -1.0, 1.0, ALU.mult, ALU.add)

            # mask assembly / colsum / rownorm / DMA in halves so the PE
            # colsum matmul and the mask DMA overlap the DVE chain
            mask_pre = outp.tile([P, n_tiles, E], f32)
            flag2w = outp.tile([P, n_tiles, E], f32)
            rowsum = outp.tile([P, n_tiles], f32)
            rinv = outp.tile([P, n_tiles], f32)
            csp = psp.tile([1, n_tiles * E], f32, tag="ps")
            mask_dr = mask_d.rearrange("(n p) e -> p n e", p=P)
            nt2 = n_tiles // 2
            for hh in range(2):
                sl = slice(hh * nt2, (hh + 1) * nt2)
                nc.vector.tensor_tensor(
                    mask_pre[:, sl, :], flag1[:, sl, :],
                    s1[:, sl].broadcast_to([P, nt2, E]), ALU.mult,
                )
                nc.vector.tensor_tensor(
                    flag2w[:, sl, :], flag2[:, sl, :],
                    s2[:, sl].broadcast_to([P, nt2, E]), ALU.mult,
                )
                nc.vector.tensor_add(
                    mask_pre[:, sl, :], mask_pre[:, sl, :], flag2w[:, sl, :]
                )
                nc.tensor.matmul(
                    csp[:, hh * nt2 * E:(hh + 1) * nt2 * E],
                    lhsT=ones_sb[:],
                    rhs=mask_pre[:, sl, :].rearrange("p n e -> p (n e)"),
                    start=True,
                    stop=True,
                )
                nc.vector.reduce_sum(
                    rowsum[:, sl], mask_pre[:, sl, :], axis=AX.X
                )
                nc.vector.tensor_scalar(
                    rowsum[:, sl], rowsum[:, sl], EPS, None, ALU.max
                )
                nc.vector.reciprocal(rinv[:, sl], rowsum[:, sl])
                nc.vector.tensor_tensor(
                    mask_pre[:, sl, :], mask_pre[:, sl, :],
                    rinv[:, sl].broadcast_to([P, nt2, E]), ALU.mult,
                )
                nc.sync.dma_start(mask_dr[:, sl, :], mask_pre[:, sl, :])

            cs_sb = outp.tile([1, E], f32)
            nc.vector.reduce_sum(
                cs_sb[:], csp[:].rearrange("p (n e) -> p e n", e=E), axis=AX.X
            )
            nc.sync.dma_start(cs_d[:, :], cs_sb[:])

    nc.compile()
    return nc


def _get_graph(n_tokens, use_c, n_cores=8):
    key = (n_tokens, use_c, n_cores)
    if key not in _GRAPH_CACHE:
        _GRAPH_CACHE[key] = build_graph(n_tokens, use_c, n_cores)
    return _GRAPH_CACHE[key]


def _host_fallback(x, ln_w, ln_b, w1, b1, w2, b2, top_k, num_experts):
    """Full numpy reference path; only used if the capacity drop binds."""
    B, S, Hd = x.shape
    N = B * S
    mu = x.mean(-1, keepdims=True)
    var = ((x - mu) ** 2).mean(-1, keepdims=True)
    h = (x - mu) / np.sqrt(var + LN_EPS) * ln_w + ln_b
    pre = h @ w1.T + b1
    from numpy import vectorize
    from math import erf as _erf
    h2 = pre * 0.5 * (1.0 + np.vectorize(_erf)(pre / math.sqrt(2.0)))
    logits = h2 @ w2.T + b2
    ew = np.clip(logits / TEMP, -50.0, 50.0).astype(np.float32)
    flat = ew.reshape(N, num_experts)
    capacity = int(CAP_FACTOR * N * top_k / num_experts)
    tk_i = np.argsort(-flat, axis=-1, kind="stable")[:, :top_k]
    tk_w = np.take_along_axis(flat, tk_i, axis=-1)
    ex = np.exp(tk_w - tk_w.max(-1, keepdims=True))
    sm = (ex / ex.sum(-1, keepdims=True)).astype(np.float32)
    masks = np.zeros((N, num_experts), np.float32)
    np.put_along_axis(masks, tk_i, sm, axis=-1)
    cols = masks.T.copy()
    sums = cols.sum(-1)
    kcap = min(capacity, N)
    over = sums > capacity
    for e in np.where(over)[0]:
        order = np.argsort(-cols[e], kind="stable")
        keep = order[:kcap]
        ncol = np.zeros_like(cols[e])
        ncol[keep] = cols[e][keep]
        cols[e] = ncol
    count = np.where(over, cols.sum(-1), sums)
    masks = cols.T
    masks = masks / np.maximum(masks.sum(-1, keepdims=True), EPS)
    usage = (count / np.maximum(count.sum(), EPS)).astype(np.float32)
    target = np.full((num_experts,), 1.0 / num_experts, np.float32)
    kl = np.sum(target * (np.log(target) - np.log(np.maximum(usage, EPS)))) / num_experts
    loss = np.float32(0.01 * kl)
    return (
        ew.reshape(B, S, num_experts),
        masks.reshape(B, S, num_experts).astype(np.float32),
        loss,
        usage,
    )


def _prep_inputs(x, ln_w, ln_b, w1, b1, w2, b2, n_cores=8):
    """Host-side weight layout prep + sharding."""
    w1s = np.ascontiguousarray((w1 * ln_w[None, :]).T).astype(np.float16)  # [H, H2]
    cvec = (b1 + w1 @ ln_b).astype(np.float32)                        # [H2]
    use_c = bool(np.any(cvec))
    w2s = np.ascontiguousarray(w2.T, np.float32)                      # [H2, E]
    w2h = w2s.astype(np.float16)
    w2l = ((w2s - w2h.astype(np.float32)) * 4096.0).astype(np.float16)
    w2hl = np.ascontiguousarray(np.concatenate(
        [w2h, np.zeros((w2h.shape[0], 24), np.float16), w2l], axis=1))
    b2s = np.ascontiguousarray(b2.reshape(E, 1), np.float32)
    ident = np.eye(P, dtype=np.float32)
    xs = x.reshape(n_cores, -1, H)
    in_maps = []
    for c in range(n_cores):
        m = {
            "x": np.ascontiguousarray(xs[c], np.float32),
            "w1s": w1s,
            "w2hl": w2hl,
            "b2s": b2s,
            "ident": ident,
            "identh": ident.astype(np.float16),
        }
        if use_c:
            m["cvec"] = cvec.reshape(H2, 1)
        in_maps.append(m)
    return in_maps, use_c


def _finalize(results, B, S, n_cores=8):
    """Gather shards; global column-sum reduce; usage + loss on host."""
    N = B * S
    num_experts = E
    ew = np.stack([results[c]["ew"] for c in range(n_cores)]).reshape(B, S, E)
    masks = np.stack([results[c]["mask"] for c in range(n_cores)]).reshape(B, S, E)
    cs = np.stack([results[c]["cs"][0] for c in range(n_cores)]).sum(axis=0)
    count = cs.astype(np.float32)
    usage = (count / np.maximum(count.sum(), EPS)).astype(np.float32)
    target = np.full((num_experts,), 1.0 / num_experts, np.float32)
    kl = np.sum(
        target * (np.log(target) - np.log(np.maximum(usage, EPS)))
    ) / num_experts
    loss = np.float32(0.01 * kl)
    return ew, masks, loss, usage, cs


def run(x, ln_w, ln_b, w1, b1, w2, b2, top_k, num_experts, trace=False):
    assert int(top_k) == 2 and int(num_experts) == E
    B, S, Hd = x.shape
    assert Hd == H and B == 8 and S == 4096, (B, S, Hd)
    n_cores = 8
    n_tokens = B * S // n_cores
    x = np.asarray(x, np.float32)
    in_maps, use_c = _prep_inputs(
        x, np.asarray(ln_w, np.float32), np.asarray(ln_b, np.float32),
        np.asarray(w1, np.float32), np.asarray(b1, np.float32),
        np.asarray(w2, np.float32), np.asarray(b2, np.float32), n_cores
    )
    nc = _get_graph(n_tokens, use_c, n_cores)
    res = run_bass_kernel_spmd(
        nc, in_maps, core_ids=list(range(n_cores)), trace=trace
    )
    ew, masks, loss, usage, cs = _finalize(res.results, B, S, n_cores)
    capacity = int(CAP_FACTOR * B * S * int(top_k) / int(num_experts))
    if (cs > capacity).any():
        return _host_fallback(
            x, ln_w, ln_b, w1, b1, w2, b2, int(top_k), int(num_experts)
        ), res
    return (ew, masks, loss, usage), res


def kernel(x, ln_w, ln_b, w1, b1, w2, b2, top_k, num_experts):
    out, _ = run(x, ln_w, ln_b, w1, b1, w2, b2, top_k, num_experts)
    return out
